# revision 2
# baseline (speedup 1.0000x reference)
"""InfoNCE loss kernel for Trainium2, 8 NeuronCores — lean symmetric version.

Host prep (free, numpy): L2-normalize the 8192x512 embeddings in float64,
scale by ALPHA=16 and cast to fp8 e4m3, stage d-major per 512-row group;
positives and the self-similarity terms are also float64 host work.

Device (per core, identical program): 17 of the 136 unique 512x512 blocks
of the symmetric similarity matrix. Core k owns row-groups k and k+8; its
pairs are (slot 0, r=0..8) and (slot 8, r=8..15) over slots s -> group
(k+s)%16, which covers every unordered group pair exactly once across the
8 cores. Per pair-group (<=4 pairs sharing the lhs slot):
  - 8 fp8 DoubleRow matmuls per row-subtile ii fill a [128, W, 512] PSUM
    tile (W pairs x 1 bank), double-buffered 4+4 banks;
  - one ACT Exp per ii covers all W pairs, writes fp8 to SBUF, and its
    accum_out yields the row-sum partial for free;
  - column sums: per pair, 2 DoubleRow matmuls with indicator-column fp8
    weights (ones in column t only) accumulate each pair's colsum onto a
    distinct partition row of one PSUM bank time-shared with the pm pool;
    a single [4,512] DVE copy drains 4 pairs.
Inputs arrive via HWDGE (sync-engine) per-slot DMAs in consumption order;
a few throwaway matmuls warm the PE clock gate and a dummy Exp preloads
the activation table while the first DMA is in flight.
"""

import numpy as np
import ml_dtypes

B = 4096
D = 512
N = 2 * B
NCORES = 8
P = 128
NT = 512          # rows per group-subblock column dim
NG = 16           # row groups of 512
GS = N // NG      # 512
CTILES = D // P   # 4
INV_T = 2.0
ALPHA = 16.0
EXP_SCALE = INV_T / (ALPHA * ALPHA)

# pair-groups: (lhs slot, rhs slots). Slot s of core k holds group (k+s)%16.
GROUPS = [
    (0, (0, 1, 2, 3)),
    (0, (4, 5, 6, 7)),
    (0, (8,)),
    (8, (8, 9, 10, 11)),
    (8, (12, 13, 14, 15)),
]
NGRP = len(GROUPS)

_CACHE = {}


def _build_bass():
    import concourse.bass as bass  # noqa: F401
    import concourse.tile as tile
    from concourse import bacc, mybir
    from contextlib import ExitStack

    dt = mybir.dt
    AF = mybir.ActivationFunctionType
    DR = mybir.MatmulPerfMode.DoubleRow

    nc = bacc.Bacc(None, target_bir_lowering=False, debug=False, num_swdge_queues=4)

    # -------- DRAM I/O --------
    # zt: slot s = fp8(ALPHA * z_norm) of group (k+s)%16, d-major:
    # zt[s][p][c][j] = zq[group_row j, c*128+p]
    zt_d = nc.dram_tensor("zt", [NG, P, CTILES, NT], dt.float8e4,
                          kind="ExternalInput")
    rs_d = nc.dram_tensor("rowsums", [P, NGRP, 4], dt.float32,
                          kind="ExternalOutput")
    cs_d = nc.dram_tensor("colsums", [4, NGRP, NT], dt.float32,
                          kind="ExternalOutput")

    with tile.TileContext(nc) as tc, ExitStack() as ctx:
        const = ctx.enter_context(tc.tile_pool(name="const", bufs=1))
        persist = ctx.enter_context(tc.tile_pool(name="persist", bufs=1))
        ejp = ctx.enter_context(tc.tile_pool(name="ejp", bufs=2))
        psum = ctx.enter_context(tc.tile_pool(name="psum", bufs=2, space="PSUM"))

        # constants / scratch
        scratch = const.tile([P, NT], dt.bfloat16)
        nc.vector.memset(scratch, 0.0)
        # indicator-column DoubleRow weights: onesind[:, :, t, c] = (c == t)
        onesind = const.tile([P, 2, 4, 4], dt.float8e4)
        nc.vector.memset(onesind, 0.0)
        for t in range(4):
            nc.vector.memset(onesind[:, :, t, t:t + 1], 1.0)

        # preload the exp activation table while DMA is in flight
        tblw = const.tile([P, 1], dt.float32)
        nc.scalar.activation(tblw, scratch[:, 0:1], AF.Exp, scale=EXP_SCALE)

        zs_f = persist.tile([P, NG, CTILES, NT], dt.float8e4)
        rowpart = persist.tile([P, NGRP, 4], dt.float32)
        cs_stage = persist.tile([4, NGRP, NT], dt.float32)

        # input DMAs in consumption order (HWDGE on sync, FIFO)
        for s in range(NG):
            nc.sync.dma_start(out=zs_f[:, s], in_=zt_d[s])

        # PE clock-gate warmup: throwaway matmuls on zeroed scratch
        warm = psum.tile([1, NT], dt.float32, name="warm", tag="pm")
        for w in range(3):
            nc.tensor.matmul(warm, scratch[:, 0:1], scratch,
                             start=(w == 0), stop=(w == 2))

        for gi, (l, rs) in enumerate(GROUPS):
            W = len(rs)
            ej = ejp.tile([P, CTILES, W, NT], dt.float8e4,
                          name=f"ej{gi}", tag="ej")
            for ii in range(4):
                pm = psum.tile([P, W, NT], dt.float32,
                               name=f"pm{gi}_{ii}", tag="pm")
                for t, r in enumerate(rs):
                    for cc in range(2):
                        nc.tensor.matmul(
                            pm[:, t, :],
                            zs_f[:, l, 2 * cc:2 * cc + 2, ii * P:(ii + 1) * P],
                            zs_f[:, r, 2 * cc:2 * cc + 2, :],
                            start=(cc == 0), stop=(cc == 1),
                            perf_mode=DR)
                nc.scalar.activation(ej[:, ii], pm, AF.Exp, scale=EXP_SCALE,
                                     accum_out=rowpart[:, gi, ii:ii + 1])
            cspairs = [t for t, r in enumerate(rs) if r != l]
            if cspairs:
                pc = psum.tile([4, NT], dt.float32, name=f"pc{gi}", tag="pm")
                nmm = 2 * len(cspairs)
                i = 0
                for t in cspairs:
                    for iip in range(2):
                        nc.tensor.matmul(
                            pc, onesind[:, :, t, :],
                            ej[:, 2 * iip:2 * iip + 2, t, :],
                            start=(i == 0), stop=(i == nmm - 1),
                            perf_mode=DR)
                        i += 1
                nc.vector.tensor_copy(cs_stage[:, gi, :], pc)

        nc.sync.dma_start(out=rs_d[:], in_=rowpart)
        nc.sync.dma_start(out=cs_d[:], in_=cs_stage)

    nc.compile()
    return nc


def _get_nc():
    if "nc" not in _CACHE:
        _CACHE["nc"] = _build_bass()
    return _CACHE["nc"]


def _prep_inputs(polyline_embs, c_embs):
    fp8 = ml_dtypes.float8_e4m3fn
    z = np.concatenate([np.asarray(polyline_embs, np.float64),
                        np.asarray(c_embs, np.float64)], axis=0)  # [8192, 512]
    z = z / np.maximum(np.linalg.norm(z, axis=1, keepdims=True), 1e-12)

    zq8 = (z * ALPHA).astype(fp8)                 # [8192, 512] fp8
    zq = zq8.astype(np.float64)
    # positives (float64, exact vs reference)
    pos = np.concatenate([np.einsum("ij,ij->i", z[:B], z[B:]),
                          np.einsum("ij,ij->i", z[B:], z[:B])])
    # self-similarity term included in diagonal-block rowsums
    self_term = np.exp(EXP_SCALE * np.einsum("ij,ij->i", zq, zq))

    xt = np.ascontiguousarray(zq8.T)              # [512, 8192] fp8
    gtiles = []
    for g in range(NG):
        t = xt[:, g * GS:(g + 1) * GS].reshape(CTILES, P, NT).transpose(1, 0, 2)
        gtiles.append(np.ascontiguousarray(t))    # [128, 4, 512]

    in_maps = []
    for k in range(NCORES):
        zt = np.stack([gtiles[(k + s) % NG] for s in range(NG)])
        in_maps.append({"zt": zt})
    return in_maps, pos, self_term


def _combine(results, pos, self_term):
    denom = np.zeros(N, np.float64)
    for k, r in enumerate(results):
        rp = r["rowsums"].astype(np.float64)      # [128, NGRP, 4]
        cs = r["colsums"].astype(np.float64)      # [4, NGRP, 512]
        for gi, (l, rs) in enumerate(GROUPS):
            ga = (k + l) % NG
            for ii in range(4):
                base = ga * GS + ii * P
                denom[base:base + P] += rp[:, gi, ii]
            for t, rr in enumerate(rs):
                if rr == l:
                    continue
                gb = (k + rr) % NG
                denom[gb * GS:(gb + 1) * GS] += cs[t, gi, :]
    denom -= self_term
    loss = np.mean(np.log(denom) - INV_T * pos)
    return np.float32(loss), denom, pos


def kernel(polyline_embs, c_embs):
    from concourse.bass_utils import run_bass_kernel_spmd

    nc = _get_nc()
    in_maps, pos, self_term = _prep_inputs(polyline_embs, c_embs)
    res = run_bass_kernel_spmd(nc, in_maps, core_ids=list(range(NCORES)))
    _CACHE["last_results"] = res
    loss, denom, _ = _combine(res.results, pos, self_term)
    _CACHE["last_denom"] = denom
    _CACHE["last_pos"] = pos
    return loss


# revision 3
# speedup vs baseline: 1.0091x; 1.0091x over previous
"""InfoNCE loss kernel for Trainium2, 8 NeuronCores — lean symmetric version.

Host prep (free, numpy): L2-normalize the 8192x512 embeddings in float64,
scale by ALPHA=16 and cast to fp8 e4m3, stage d-major per 512-row group;
positives and the self-similarity terms are also float64 host work.

Device (per core, identical program): 17 of the 136 unique 512x512 blocks
of the symmetric similarity matrix. Core k owns row-groups k and k+8; its
pairs are (slot 0, r=0..8) and (slot 8, r=8..15) over slots s -> group
(k+s)%16, which covers every unordered group pair exactly once across the
8 cores. Pairs run in 6 groups of <=3 sharing the lhs slot:
  - 6 fp8 DoubleRow matmuls per row-subtile ii fill a [128, W, 512] PSUM
    tile (W pairs x 1 bank), double-buffered 3+3 banks;
  - one ACT Exp per ii covers all W pairs, writes fp8 to SBUF, and its
    accum_out yields the row-sum partial for free;
  - column sums: per pair, 2 DoubleRow matmuls with indicator-column fp8
    weights (ones in column t only) accumulate each pair's colsum onto a
    distinct partition row of a dedicated double-buffered PSUM bank; one
    [4,512] DVE copy drains a whole group. Colsum emission is software-
    pipelined one group late so it never head-of-line blocks the PE on
    the group's last ACT.
4 fill tiles per group keep the pm double-buffer opposite-parity across
group boundaries (no fill-vs-ACT collision). Inputs arrive via HWDGE
(sync-engine) per-slot DMAs in consumption order; a few throwaway matmuls
warm the PE clock gate and a dummy Exp preloads the activation table
while the first DMA is in flight.
"""

import numpy as np
import ml_dtypes

B = 4096
D = 512
N = 2 * B
NCORES = 8
P = 128
NT = 512          # block column dim
NG = 16           # row groups of 512
GS = N // NG      # 512
CTILES = D // P   # 4
INV_T = 2.0
ALPHA = 16.0
EXP_SCALE = INV_T / (ALPHA * ALPHA)

# pair-groups: (lhs slot, rhs slots). Slot s of core k holds group (k+s)%16.
GROUPS = [
    (0, (0, 1, 2)),
    (0, (3, 4, 5)),
    (0, (6, 7, 8)),
    (8, (8, 9, 10)),
    (8, (11, 12, 13)),
    (8, (14, 15)),
]
NGRP = len(GROUPS)

_CACHE = {}


def _build_bass():
    import concourse.bass as bass  # noqa: F401
    import concourse.tile as tile
    from concourse import bacc, mybir
    from contextlib import ExitStack

    dt = mybir.dt
    AF = mybir.ActivationFunctionType
    DR = mybir.MatmulPerfMode.DoubleRow

    nc = bacc.Bacc(None, target_bir_lowering=False, debug=False, num_swdge_queues=4)

    # -------- DRAM I/O --------
    # zt: slot s = fp8(ALPHA * z_norm) of group (k+s)%16, d-major:
    # zt[s][p][c][j] = zq[group_row j, c*128+p]
    zt_d = nc.dram_tensor("zt", [NG, P, CTILES, NT], dt.float8e4,
                          kind="ExternalInput")
    rs_d = nc.dram_tensor("rowsums", [P, NGRP, 4], dt.float32,
                          kind="ExternalOutput")
    cs_d = nc.dram_tensor("colsums", [4, NGRP, NT], dt.float32,
                          kind="ExternalOutput")

    with tile.TileContext(nc) as tc, ExitStack() as ctx:
        const = ctx.enter_context(tc.tile_pool(name="const", bufs=1))
        persist = ctx.enter_context(tc.tile_pool(name="persist", bufs=1))
        ejp = ctx.enter_context(tc.tile_pool(name="ejp", bufs=2))
        psum = ctx.enter_context(tc.tile_pool(name="psum", bufs=2, space="PSUM"))

        # constants / scratch
        actw = const.tile([P, 1], dt.bfloat16)
        nc.gpsimd.memset(actw, 0.0)
        scratch = const.tile([P, NT], dt.bfloat16)
        nc.vector.memset(scratch, 0.0)
        # indicator-column DoubleRow weights: onesind[:, :, t, c] = (c == t)
        onesind = const.tile([P, 2, 4, 4], dt.float8e4)
        nc.vector.memset(onesind, 0.0)
        for t in range(4):
            nc.vector.memset(onesind[:, :, t, t:t + 1], 1.0)

        # preload the exp activation table while DMA is in flight
        tblw = const.tile([P, 1], dt.float32)
        nc.scalar.activation(tblw, actw, AF.Exp, scale=EXP_SCALE)

        zs_f = persist.tile([P, NG, CTILES, NT], dt.float8e4)
        rowpart = persist.tile([P, NGRP, 4], dt.float32)
        cs_stage = persist.tile([4, NGRP, NT], dt.float32)

        # input DMAs in consumption order (HWDGE on sync, FIFO)
        for s in range(NG):
            nc.sync.dma_start(out=zs_f[:, s], in_=zt_d[s])

        # PE clock-gate warmup: throwaway matmuls on zeroed scratch
        warm = psum.tile([1, NT], dt.float32, name="warm", tag="pm")
        for w in range(4):
            nc.tensor.matmul(warm, scratch[:, 0:1], scratch,
                             start=(w == 0), stop=(w == 3))

        def emit_cs(gi, l, rs, ej):
            cspairs = [t for t, r in enumerate(rs) if r != l]
            pc = psum.tile([4, NT], dt.float32, name=f"pc{gi}", tag="pc")
            nmm = 2 * len(cspairs)
            i = 0
            for t in cspairs:
                for iip in range(2):
                    nc.tensor.matmul(
                        pc, onesind[:, :, t, :],
                        ej[:, 2 * iip:2 * iip + 2, t, :],
                        start=(i == 0), stop=(i == nmm - 1),
                        perf_mode=DR)
                    i += 1
            nc.vector.tensor_copy(cs_stage[:, gi, :], pc)
            nc.sync.dma_start(out=cs_d[:, gi], in_=cs_stage[:, gi, :])

        pending = None
        for gi, (l, rs) in enumerate(GROUPS):
            W = len(rs)
            ej = ejp.tile([P, CTILES, W, NT], dt.float8e4,
                          name=f"ej{gi}", tag="ej")
            for ii in range(4):
                pm = psum.tile([P, W, NT], dt.float32,
                               name=f"pm{gi}_{ii}", tag="pm")
                for t, r in enumerate(rs):
                    for cc in range(2):
                        nc.tensor.matmul(
                            pm[:, t, :],
                            zs_f[:, l, 2 * cc:2 * cc + 2, ii * P:(ii + 1) * P],
                            zs_f[:, r, 2 * cc:2 * cc + 2, :],
                            start=(cc == 0), stop=(cc == 1),
                            perf_mode=DR)
                nc.scalar.activation(ej[:, ii], pm, AF.Exp, scale=EXP_SCALE,
                                     accum_out=rowpart[:, gi, ii:ii + 1])
                if ii == 2 and pending is not None:
                    emit_cs(*pending)
                    pending = None
            pending = (gi, l, rs, ej)
        emit_cs(*pending)

        nc.sync.dma_start(out=rs_d[:], in_=rowpart)

    nc.compile()
    return nc


def _get_nc():
    if "nc" not in _CACHE:
        _CACHE["nc"] = _build_bass()
    return _CACHE["nc"]


def _prep_inputs(polyline_embs, c_embs):
    fp8 = ml_dtypes.float8_e4m3fn
    z = np.concatenate([np.asarray(polyline_embs, np.float64),
                        np.asarray(c_embs, np.float64)], axis=0)  # [8192, 512]
    z = z / np.maximum(np.linalg.norm(z, axis=1, keepdims=True), 1e-12)

    zq8 = (z * ALPHA).astype(fp8)                 # [8192, 512] fp8
    zq = zq8.astype(np.float64)
    # positives (float64, exact vs reference)
    pos = np.concatenate([np.einsum("ij,ij->i", z[:B], z[B:]),
                          np.einsum("ij,ij->i", z[B:], z[:B])])
    # self-similarity term included in diagonal-block rowsums
    self_term = np.exp(EXP_SCALE * np.einsum("ij,ij->i", zq, zq))

    xt = np.ascontiguousarray(zq8.T)              # [512, 8192] fp8
    gtiles = []
    for g in range(NG):
        t = xt[:, g * GS:(g + 1) * GS].reshape(CTILES, P, NT).transpose(1, 0, 2)
        gtiles.append(np.ascontiguousarray(t))    # [128, 4, 512]

    in_maps = []
    for k in range(NCORES):
        zt = np.stack([gtiles[(k + s) % NG] for s in range(NG)])
        in_maps.append({"zt": zt})
    return in_maps, pos, self_term


def _combine(results, pos, self_term):
    denom = np.zeros(N, np.float64)
    for k, r in enumerate(results):
        rp = r["rowsums"].astype(np.float64)      # [128, NGRP, 4]
        cs = r["colsums"].astype(np.float64)      # [4, NGRP, 512]
        for gi, (l, rs) in enumerate(GROUPS):
            ga = (k + l) % NG
            for ii in range(4):
                base = ga * GS + ii * P
                denom[base:base + P] += rp[:, gi, ii]
            for t, rr in enumerate(rs):
                if rr == l:
                    continue
                gb = (k + rr) % NG
                denom[gb * GS:(gb + 1) * GS] += cs[t, gi, :]
    denom -= self_term
    loss = np.mean(np.log(denom) - INV_T * pos)
    return np.float32(loss), denom, pos


def kernel(polyline_embs, c_embs):
    from concourse.bass_utils import run_bass_kernel_spmd

    nc = _get_nc()
    in_maps, pos, self_term = _prep_inputs(polyline_embs, c_embs)
    res = run_bass_kernel_spmd(nc, in_maps, core_ids=list(range(NCORES)))
    _CACHE["last_results"] = res
    loss, denom, _ = _combine(res.results, pos, self_term)
    _CACHE["last_denom"] = denom
    _CACHE["last_pos"] = pos
    return loss


# revision 4
# speedup vs baseline: 1.0655x; 1.0559x over previous
"""InfoNCE loss kernel for Trainium2, 8 NeuronCores — lean symmetric version.

Host prep (free, numpy): L2-normalize the 8192x512 embeddings in float64,
scale by ALPHA=16 and cast to fp8 e4m3, stage d-major per 512-row group;
positives and the self-similarity terms are also float64 host work.

Device (per core, identical program): 17 of the 136 unique 512x512 blocks
of the symmetric similarity matrix. Core k owns row-groups k and k+8; its
pairs are (slot 0, r=0..8) and (slot 8, r=8..15) over slots s -> group
(k+s)%16, which covers every unordered group pair exactly once across the
8 cores. Pairs run in 5 groups sized (1,4,4,4,4) sharing the lhs slot;
the W=1 diagonal group goes first, needing only slot 0, so its ACTs fill
the DMA-ramp window (it also has no column sums).
  - 2 fp8 DoubleRow matmuls per (pair, row-subtile ii) fill a
    [128, W, 512] PSUM tile (W pairs x 1 bank), double-buffered 4+4 banks;
  - one ACT Exp per ii covers all W pairs (the even width keeps the ACT
    datapath on its fast ~0.8 ns/el path), writes fp8 ej to SBUF, and its
    accum_out yields the row-sum partial for free;
  - column sums per pair: 2 DoubleRow matmuls with indicator-column fp8
    weights (ones in column t only) put each pair's colsum on its own
    partition row. They are split into two half-tiles pca (ej ii0-1,
    ready after ACT-ii1) and pcb (ej ii2-3, after ACT-ii3) that ride the
    pm pool rotation — 6 tiles/group keeps the double-buffer parity
    opposite across group boundaries — with pcb software-pipelined into
    the next group's fill stream so it never head-of-line blocks the PE.
Inputs arrive via HWDGE (sync-engine) per-slot DMAs in consumption order;
a few throwaway matmuls warm the PE clock gate and a dummy Exp preloads
the activation table while the first DMA is in flight.
"""

import numpy as np
import ml_dtypes

B = 4096
D = 512
N = 2 * B
NCORES = 8
P = 128
NT = 512          # block column dim
NG = 16           # row groups of 512
GS = N // NG      # 512
CTILES = D // P   # 4
INV_T = 2.0
ALPHA = 16.0
EXP_SCALE = INV_T / (ALPHA * ALPHA)

# pair-groups: (lhs slot, rhs slots). Slot s of core k holds group (k+s)%16.
GROUPS = [
    (0, (0,)),
    (0, (1, 2, 3, 4)),
    (0, (5, 6, 7, 8)),
    (8, (8, 9, 10, 11)),
    (8, (12, 13, 14, 15)),
]
NGRP = len(GROUPS)

_CACHE = {}


def _build_bass():
    import concourse.bass as bass  # noqa: F401
    import concourse.tile as tile
    from concourse import bacc, mybir
    from contextlib import ExitStack

    dt = mybir.dt
    AF = mybir.ActivationFunctionType
    DR = mybir.MatmulPerfMode.DoubleRow

    nc = bacc.Bacc(None, target_bir_lowering=False, debug=False, num_swdge_queues=4)

    # -------- DRAM I/O --------
    # zt: slot s = fp8(ALPHA * z_norm) of group (k+s)%16, d-major:
    # zt[s][p][c][j] = zq[group_row j, c*128+p]
    zt_d = nc.dram_tensor("zt", [NG, P, CTILES, NT], dt.float8e4,
                          kind="ExternalInput")
    rs_d = nc.dram_tensor("rowsums", [P, NGRP, 4], dt.float32,
                          kind="ExternalOutput")
    cs_d = nc.dram_tensor("colsums", [4, NGRP, 2, NT], dt.float32,
                          kind="ExternalOutput")

    with tile.TileContext(nc) as tc, ExitStack() as ctx:
        const = ctx.enter_context(tc.tile_pool(name="const", bufs=1))
        persist = ctx.enter_context(tc.tile_pool(name="persist", bufs=1))
        ejp = ctx.enter_context(tc.tile_pool(name="ejp", bufs=2))
        psum = ctx.enter_context(tc.tile_pool(name="psum", bufs=2, space="PSUM"))

        # constants / scratch
        actw = const.tile([P, 1], dt.bfloat16)
        nc.gpsimd.memset(actw, 0.0)
        scratch = const.tile([P, NT], dt.bfloat16)
        nc.vector.memset(scratch, 0.0)
        # indicator-column DoubleRow weights: onesind[:, :, t, c] = (c == t)
        onesind = const.tile([P, 2, 4, 4], dt.float8e4)
        nc.vector.memset(onesind, 0.0)
        for t in range(4):
            nc.vector.memset(onesind[:, :, t, t:t + 1], 1.0)

        # preload the exp activation table while DMA is in flight
        tblw = const.tile([P, 1], dt.float32)
        nc.scalar.activation(tblw, actw, AF.Exp, scale=EXP_SCALE)

        zs_f = persist.tile([P, NG, CTILES, NT], dt.float8e4)
        rowpart = persist.tile([P, NGRP, 4], dt.float32)
        cs_stage = persist.tile([4, NGRP, 2, NT], dt.float32)

        # input DMAs in consumption order (HWDGE on sync, FIFO)
        for s in range(NG):
            nc.sync.dma_start(out=zs_f[:, s], in_=zt_d[s])

        # PE clock-gate warmup: throwaway matmuls on zeroed scratch
        warm = psum.tile([1, NT], dt.float32, name="warm", tag="pm")
        for w in range(4):
            nc.tensor.matmul(warm, scratch[:, 0:1], scratch,
                             start=(w == 0), stop=(w == 3))

        def emit_cs_half(gi, l, rs, ej, half):
            """Colsum half: 1 DR matmul per pair over ej ii slices
            [2*half, 2*half+1], each pair on its own partition row."""
            cspairs = [t for t, r in enumerate(rs) if r != l]
            pc = psum.tile([4, NT], dt.float32,
                           name=f"pc{gi}_{half}", tag="pm")
            for i, t in enumerate(cspairs):
                nc.tensor.matmul(
                    pc, onesind[:, :, t, :],
                    ej[:, 2 * half:2 * half + 2, t, :],
                    start=(i == 0), stop=(i == len(cspairs) - 1),
                    perf_mode=DR)
            nc.vector.tensor_copy(cs_stage[:, gi, half, :], pc)

        pend_b = None  # (gi, l, rs, ej) awaiting its second colsum half
        for gi, (l, rs) in enumerate(GROUPS):
            W = len(rs)
            ej = ejp.tile([P, CTILES, W, NT], dt.float8e4,
                          name=f"ej{gi}", tag="ej")
            for ii in range(4):
                if ii == 1 and pend_b is not None:
                    emit_cs_half(*pend_b, half=1)
                    pend_b = None
                pm = psum.tile([P, W, NT], dt.float32,
                               name=f"pm{gi}_{ii}", tag="pm")
                for t, r in enumerate(rs):
                    for cc in range(2):
                        nc.tensor.matmul(
                            pm[:, t, :],
                            zs_f[:, l, 2 * cc:2 * cc + 2, ii * P:(ii + 1) * P],
                            zs_f[:, r, 2 * cc:2 * cc + 2, :],
                            start=(cc == 0), stop=(cc == 1),
                            perf_mode=DR)
                nc.scalar.activation(ej[:, ii], pm, AF.Exp, scale=EXP_SCALE,
                                     accum_out=rowpart[:, gi, ii:ii + 1])
            if any(r != l for r in rs):
                emit_cs_half(gi, l, rs, ej, half=0)
                pend_b = (gi, l, rs, ej)
        emit_cs_half(*pend_b, half=1)

        nc.sync.dma_start(out=rs_d[:], in_=rowpart)
        nc.sync.dma_start(out=cs_d[:], in_=cs_stage)

    nc.compile()
    return nc


def _get_nc():
    if "nc" not in _CACHE:
        _CACHE["nc"] = _build_bass()
    return _CACHE["nc"]


def _prep_inputs(polyline_embs, c_embs):
    fp8 = ml_dtypes.float8_e4m3fn
    z = np.concatenate([np.asarray(polyline_embs, np.float64),
                        np.asarray(c_embs, np.float64)], axis=0)  # [8192, 512]
    z = z / np.maximum(np.linalg.norm(z, axis=1, keepdims=True), 1e-12)

    zq8 = (z * ALPHA).astype(fp8)                 # [8192, 512] fp8
    zq = zq8.astype(np.float64)
    # positives (float64, exact vs reference)
    pos = np.concatenate([np.einsum("ij,ij->i", z[:B], z[B:]),
                          np.einsum("ij,ij->i", z[B:], z[:B])])
    # self-similarity term included in diagonal-block rowsums
    self_term = np.exp(EXP_SCALE * np.einsum("ij,ij->i", zq, zq))

    xt = np.ascontiguousarray(zq8.T)              # [512, 8192] fp8
    gtiles = []
    for g in range(NG):
        t = xt[:, g * GS:(g + 1) * GS].reshape(CTILES, P, NT).transpose(1, 0, 2)
        gtiles.append(np.ascontiguousarray(t))    # [128, 4, 512]

    in_maps = []
    for k in range(NCORES):
        zt = np.stack([gtiles[(k + s) % NG] for s in range(NG)])
        in_maps.append({"zt": zt})
    return in_maps, pos, self_term


def _combine(results, pos, self_term):
    denom = np.zeros(N, np.float64)
    for k, r in enumerate(results):
        rp = r["rowsums"].astype(np.float64)      # [128, NGRP, 4]
        cs = r["colsums"].astype(np.float64)      # [4, NGRP, 2, 512]
        for gi, (l, rs) in enumerate(GROUPS):
            ga = (k + l) % NG
            for ii in range(4):
                base = ga * GS + ii * P
                denom[base:base + P] += rp[:, gi, ii]
            csp = 0
            for t, rr in enumerate(rs):
                if rr == l:
                    continue
                gb = (k + rr) % NG
                denom[gb * GS:(gb + 1) * GS] += cs[csp, gi, 0] + cs[csp, gi, 1]
                csp += 1
    denom -= self_term
    loss = np.mean(np.log(denom) - INV_T * pos)
    return np.float32(loss), denom, pos


def kernel(polyline_embs, c_embs):
    from concourse.bass_utils import run_bass_kernel_spmd

    nc = _get_nc()
    in_maps, pos, self_term = _prep_inputs(polyline_embs, c_embs)
    res = run_bass_kernel_spmd(nc, in_maps, core_ids=list(range(NCORES)))
    _CACHE["last_results"] = res
    loss, denom, _ = _combine(res.results, pos, self_term)
    _CACHE["last_denom"] = denom
    _CACHE["last_pos"] = pos
    return loss
